# revision 32
# baseline (speedup 1.0000x reference)
"""CPMAnt transformer block on 8 TRN2 NeuronCores (Megatron-style TP).

Core c owns 4 attention heads and 1280 FFN columns. Activations are
feature-major (D on partitions). QKV / attention-out / AV / softmax-sum /
sum-of-squares matmuls run in fp8 (e4m3 / e5m2) DoubleRow mode (2 k-tiles
per instruction = 2x PE throughput); scores and the FFN run in bf16.
q/k/v and attention probabilities never leave SBUF. Scores are computed
k-major (out[k, q]) so no PE transposes are needed; position bias is
stored e4m3 and added to the score PSUM on the DVE (no eye-matmul); the
softmax denominator comes from an fp8 ones-matmul and normalization is
folded into the attn output copy.

DMA-traffic cuts vs the first version: pb e4m3 (16.8MB), wq/wk/wv
streamed once per chunk-pair (12MB), wo streamed once (6MB), residual
base read as bf16 hT rows (14MB). FFN prologues are split into a DVE
part (h reload + attn-sum add + squares) emitted before the previous
chunk's wout units and an MM part (variance matmuls + normalize)
emitted after, so the in-order PE queue never drains at chunk
boundaries. The last chunk's wout+ReduceScatter run in two token
halves so the tail only waits on a 2MB collective.
"""

import math

import numpy as np

S = 2048
D = 4096
H = 32
DH = 128
FF = 10240
NCORES = 8
P = 128
HPC = H // NCORES            # 4 heads per core
WPC = HPC * DH               # 512   per-core qkv width
FPC = FF // NCORES           # 1280  per-core ff width
FCC = FPC // P               # 10
DC = D // P                  # 32
DCH = DC // 2                # 16
SCN = 4                      # S chunks
SCW = S // SCN               # 512
SCH = SCW // 2               # 256 (tail-split width)
KC = S // P                  # 16 key chunks
EPS = 1e-6

# fp8 weight scales (powers of two; descaled at psum copy-out)
S_WQ = 256.0                 # wq folded with 1/sqrt(DH): std ~0.0014
S_WK = 16.0
S_WV = 16.0
S_WO = 16.0
S_QS = 4.0                   # q stored as 4*q (e4m3); pb pre-scaled by 4 on host
S_VS = 8.0                   # v stored as 8*v; cancels with attn fp8 scale
S_AR = 256.0                 # attn-out partials AllReduced in e4m3 at this scale
                             # (small enough that the 8-way sum stays in range)
S_RS = 64.0                  # last chunk's (ffn+attn) partials ReduceScattered
                             # in e4m3; only 1/4 of tokens see the extra noise

_CACHE = {}


def _build():
    import concourse.bacc as bacc
    import concourse.tile as tile
    from concourse import mybir

    f32 = mybir.dt.float32
    bf = mybir.dt.bfloat16
    e4 = mybir.dt.float8e4
    e5 = mybir.dt.float8e5
    AF = mybir.ActivationFunctionType
    ALU = mybir.AluOpType
    DR = mybir.MatmulPerfMode.DoubleRow
    RG = [list(range(NCORES))]

    nc = bacc.Bacc(None, num_devices=NCORES)

    hT = nc.dram_tensor("hT", [DC, P, S], bf, kind="ExternalInput")
    h_ownb = nc.dram_tensor("h_ownb", [4, P, S], bf, kind="ExternalInput")
    wq = nc.dram_tensor("wq", [P, DCH, 2, WPC], e4, kind="ExternalInput")
    wk = nc.dram_tensor("wk", [P, DCH, 2, WPC], e4, kind="ExternalInput")
    wv = nc.dram_tensor("wv", [P, DCH, 2, WPC], e4, kind="ExternalInput")
    wo = nc.dram_tensor("wo", [P, 2, 2, D], e4, kind="ExternalInput")
    pbT = nc.dram_tensor("pbT", [HPC, SCN, 2, P, 8, SCW], e4, kind="ExternalInput")
    w01 = nc.dram_tensor("w01", [FCC, P, 2, DC, P], bf, kind="ExternalInput")
    wout = nc.dram_tensor("wout", [8, P, FCC, 4 * P], bf, kind="ExternalInput")
    ones4 = nc.dram_tensor("ones4", [P, 2, P], e4, kind="ExternalInput")
    ones5 = nc.dram_tensor("ones5", [P, 2, P], e5, kind="ExternalInput")
    out = nc.dram_tensor("out", [WPC, S], bf, kind="ExternalOutput")

    from contextlib import ExitStack

    with tile.TileContext(nc) as tc:
        with ExitStack() as ctx:
            ep = ctx.enter_context
            dram = ep(tc.tile_pool(name="dram", bufs=1, space="DRAM"))
            singles = ep(tc.tile_pool(name="singles", bufs=1))
            arena = ep(tc.tile_pool(name="arena", bufs=1))
            hstr = ep(tc.tile_pool(name="hstr", bufs=3))
            xarena = ep(tc.tile_pool(name="xarena", bufs=2))
            wstr = ep(tc.tile_pool(name="wstr", bufs=2))
            pbp = ep(tc.tile_pool(name="pbp", bufs=7))
            p5p = ep(tc.tile_pool(name="p5p", bufs=2))
            stgp = ep(tc.tile_pool(name="stgp", bufs=2))
            atp = ep(tc.tile_pool(name="atp", bufs=2))
            rbp = ep(tc.tile_pool(name="rbp", bufs=2))
            psA = ep(tc.tile_pool(name="psA", bufs=4, space="PSUM"))
            psB = ep(tc.tile_pool(name="psB", bufs=4, space="PSUM"))

            # ---- DRAM scratch for collectives ----
            # attn-out partials are tiny vs the residual stream, so the
            # AllReduce runs in e4m3 (half the wire bytes of bf16)
            arin = [dram.tile([DC, P, SCW], e4, tag=f"arin{j}", name=f"arin{j}")
                    for j in range(SCN)]
            arout = [dram.tile([DC, P, SCW], e4, tag=f"arout{j}", name=f"arout{j}",
                               addr_space="Shared") for j in range(SCN)]
            rsin = [dram.tile([DC, P, SCW], bf, tag=f"rsin{j}", name=f"rsin{j}")
                    for j in range(SCN - 1)]
            rsout = [dram.tile([4, P, SCW], bf, tag=f"rsout{j}", name=f"rsout{j}")
                     for j in range(SCN - 1)]
            # last chunk: two token-half RS so the tail collective is 2MB
            rsin3 = [dram.tile([DC, P, SCH], bf, tag=f"rsin3{h}", name=f"rsin3{h}")
                     for h in range(2)]
            rsout3 = [dram.tile([4, P, SCH], bf, tag=f"rsout3{h}", name=f"rsout3{h}")
                      for h in range(2)]

            ones4_sb = singles.tile([P, 2, P], e4)
            nc.sync.dma_start(out=ones4_sb[:], in_=ones4[:])
            ones5_sb = singles.tile([P, 2, P], e5)
            nc.sync.dma_start(out=ones5_sb[:], in_=ones5[:])
            eps_t = singles.tile([P, 1], f32)
            nc.vector.memset(eps_t[:], EPS)

            # persistent SBUF arenas for q/k/v (fp8)
            qT = arena.tile([P, HPC, S], e4, tag="qT")       # [dh, h, s] = 4*q
            kT = arena.tile([P, HPC, S], e4, tag="kT")       # [dh, h, s] = k
            v8 = arena.tile([P, HPC, 8, 2, DH], e4, tag="v8")  # 8*v

            # ================= phase 1: rmsnorm1 + QKV =====================
            def qkv_norm(j):
                """hT chunk -> normalized fp8 x8; returns x8."""
                cols = slice(j * SCW, (j + 1) * SCW)
                halves = []
                ss = psB.tile([P, SCW], f32, tag="pB", name="ss_ps")
                for hf in range(2):
                    hld = hstr.tile([P, DCH, SCW], bf, tag="hstream",
                                    name=f"hld{hf}")
                    nc.sync.dma_start(
                        out=hld[:],
                        in_=hT[hf * DCH:(hf + 1) * DCH, :, cols].rearrange(
                            "d p s -> p d s"),
                    )
                    halves.append(hld)
                    sq8 = p5p.tile([P, DCH, SCW], e4, tag="p5", name=f"sq8{hf}")
                    nc.vector.tensor_mul(sq8[:], hld[:], hld[:])
                    for jj in range(DCH // 2):
                        nc.tensor.matmul(
                            ss[:], lhsT=ones4_sb[:],
                            rhs=sq8[:, 2 * jj:2 * jj + 2, :],
                            start=(hf == 0 and jj == 0),
                            stop=(hf == 1 and jj == DCH // 2 - 1),
                            perf_mode=DR,
                        )
                rbc = rbp.tile([P, SCW], f32, tag="rbc")
                nc.scalar.activation(
                    out=rbc[:], in_=ss[:], func=AF.Sqrt, bias=eps_t[:],
                    scale=1.0 / D,
                )
                nc.vector.reciprocal_approx_fast(out=rbc[:], in_=rbc[:])
                x8 = xarena.tile([P, DC, SCW], e4, tag="x8")
                for d in range(DC):
                    nc.vector.tensor_mul(
                        x8[:, d, :], halves[d // DCH][:, d % DCH, :], rbc[:])
                return x8

            def qkv_pair(j0):
                """Two S-chunks per weight stream (wq/wk/wv loaded once)."""
                x0 = qkv_norm(j0)
                # wq load issued between the two norms so it lands in time
                wqsb = wstr.tile([P, DCH, 2, WPC], e4, tag="wstream",
                                 name="wqsb")
                nc.sync.dma_start(out=wqsb[:], in_=wq[:])
                xs = [(j0, x0), (j0 + 1, qkv_norm(j0 + 1))]
                for name, wsb_pre, wsrc, dst, cscale in (
                    ("q", wqsb, wq, qT, S_QS / S_WQ),
                    ("k", None, wk, kT, 1.0 / S_WK),
                ):
                    if wsb_pre is None:
                        wsb = wstr.tile([P, DCH, 2, WPC], e4, tag="wstream",
                                        name=f"w{name}sb")
                        nc.sync.dma_start(out=wsb[:], in_=wsrc[:])
                    else:
                        wsb = wsb_pre
                    for j, x8 in xs:
                        cols = slice(j * SCW, (j + 1) * SCW)
                        for h in range(HPC):
                            ps = psA.tile([P, SCW], f32, tag="pA",
                                          name=f"ps_{name}{h}")
                            for dp in range(DCH):
                                nc.tensor.matmul(
                                    ps[:], lhsT=wsb[:, dp, :, h * DH:(h + 1) * DH],
                                    rhs=x8[:, 2 * dp:2 * dp + 2, :],
                                    start=(dp == 0), stop=(dp == DCH - 1),
                                    perf_mode=DR,
                                )
                            nc.scalar.mul(dst[:, h, cols], ps[:], cscale)

                wvsb = wstr.tile([P, DCH, 2, WPC], e4, tag="wstream", name="wvsb")
                nc.sync.dma_start(out=wvsb[:], in_=wv[:])
                for j, x8 in xs:
                    for sl in range(SCW // P):
                        ps = psA.tile([P, WPC], f32, tag="pA", name=f"ps_v{sl}")
                        for dp in range(DCH):
                            nc.tensor.matmul(
                                ps[:], lhsT=x8[:, 2 * dp:2 * dp + 2, sl * P:(sl + 1) * P],
                                rhs=wvsb[:, dp, :, :],
                                start=(dp == 0), stop=(dp == DCH - 1),
                                perf_mode=DR,
                            )
                        kcix = j * (SCW // P) + sl
                        nc.scalar.mul(
                            v8[:, :, kcix // 2, kcix % 2, :],
                            ps[:].rearrange("p (h f) -> p h f", h=HPC),
                            S_VS / S_WV,
                        )

            # ================= attention units =============================
            def attn_scores(qg, h):
                qcols = slice(qg * SCW, (qg + 1) * SCW)
                p5 = p5p.tile([P, KC, SCW], e5, tag="p5", name="p5")
                for q4 in range(4):
                    pbt = pbp.tile([P, 4, SCW], e4, tag="pbt", name="pbt")
                    nc.sync.dma_start(
                        out=pbt[:],
                        in_=pbT[h, qg, q4 // 2][:, (q4 % 2) * 4:(q4 % 2) * 4 + 4, :],
                    )
                    for kk in range(4):
                        kc = q4 * 4 + kk
                        pss = psA.tile([P, SCW], f32, tag="pA", name="pss")
                        nc.tensor.matmul(
                            pss[:], lhsT=kT[:, h, kc * P:(kc + 1) * P],
                            rhs=qT[:, h, qcols], start=True, stop=True,
                        )
                        nc.vector.tensor_add(pss[:], pss[:], pbt[:, kk, :])
                        nc.scalar.activation(
                            out=p5[:, kc, :], in_=pss[:], func=AF.Exp,
                            scale=1.0 / S_QS,
                        )
                return p5

            def attn_sums_av(qg, h, p5, attnT):
                sums = psB.tile([P, SCW], f32, tag="pB", name="sums_ps")
                for jj in range(KC // 2):
                    nc.tensor.matmul(
                        sums[:], lhsT=ones5_sb[:],
                        rhs=p5[:, 2 * jj:2 * jj + 2, :],
                        start=(jj == 0), stop=(jj == KC // 2 - 1),
                        perf_mode=DR,
                    )
                psav = psB.tile([P, SCW], f32, tag="pB", name="psav")
                for jj in range(KC // 2):
                    nc.tensor.matmul(
                        psav[:], lhsT=v8[:, h, jj, :, :],
                        rhs=p5[:, 2 * jj:2 * jj + 2, :],
                        start=(jj == 0), stop=(jj == KC // 2 - 1),
                        perf_mode=DR,
                    )
                rs = rbp.tile([P, SCW], f32, tag="rbc", name="rs")
                nc.vector.reciprocal_approx_fast(out=rs[:], in_=sums[:])
                nc.vector.tensor_mul(attnT[:, h, :], psav[:], rs[:])

            wosb = None

            def load_wo():
                nonlocal wosb
                wosb = wstr.tile([P, 2, 2, D], e4, tag="wotag", bufs=1,
                                 name="wosb")
                nc.sync.dma_start(out=wosb[:], in_=wo[:])

            def wo_unit(qg, attnT):
                for dg in range(8):
                    stg = stgp.tile([P, 4, SCW], e4, tag="stg", name="wostg")
                    for di in range(4):
                        dcc = dg * 4 + di
                        ps = psA.tile([P, SCW], f32, tag="pA", name="ps_wo")
                        for hp in range(2):
                            nc.tensor.matmul(
                                ps[:],
                                lhsT=wosb[:, hp, :, dcc * P:(dcc + 1) * P],
                                rhs=attnT[:, 2 * hp:2 * hp + 2, :],
                                start=(hp == 0), stop=(hp == 1),
                                perf_mode=DR,
                            )
                        nc.vector.tensor_scalar_mul(
                            stg[:, di, :], ps[:], S_AR / (S_VS * S_WO))
                    nc.sync.dma_start(
                        out=arin[qg][dg * 4:(dg + 1) * 4, :, :].rearrange(
                            "d p s -> p d s"),
                        in_=stg[:],
                    )
                nc.gpsimd.collective_compute(
                    "AllReduce", ALU.add, replica_groups=RG,
                    ins=[arin[qg][:]], outs=[arout[qg][:]],
                )

            def attn_units(qg):
                """List of emit-callbacks for one attention chunk."""
                attnT = atp.tile([P, HPC, SCW], e4, tag="attnT",
                                 name=f"attnT{qg}")
                state = {}

                def u_scores(h):
                    def emit():
                        p5_prev = state.get("p5")
                        state["p5"] = attn_scores(qg, h)
                        if p5_prev is not None:
                            attn_sums_av(qg, h - 1, p5_prev, attnT)
                    return emit

                def u_tail():
                    def emit():
                        attn_sums_av(qg, HPC - 1, state["p5"], attnT)
                        wo_unit(qg, attnT)
                    return emit

                return [u_scores(h) for h in range(HPC)] + [u_tail()]

            # ================= FFN units ===================================
            def ffn_pro_dve(sc):
                """h reload + attn-sum add + squares (DVE/DMA only)."""
                cols = slice(sc * SCW, (sc + 1) * SCW)
                halves, sqs = [], []
                for hf in range(2):
                    h1h = hstr.tile([P, DCH, SCW], bf, tag="hstream",
                                    name=f"h1h{hf}")
                    nc.sync.dma_start(
                        out=h1h[:],
                        in_=hT[hf * DCH:(hf + 1) * DCH, :, cols].rearrange(
                            "d p s -> p d s"),
                    )
                    for qr in range(4):
                        ars = stgp.tile([P, 4, SCW], e4, tag="arp", name="ars")
                        d0 = hf * DCH + qr * 4
                        nc.sync.dma_start(
                            out=ars[:],
                            in_=arout[sc][d0:d0 + 4, :, :].rearrange(
                                "d p s -> p d s"),
                        )
                        nc.vector.scalar_tensor_tensor(
                            out=h1h[:, qr * 4:(qr + 1) * 4, :],
                            in0=ars[:], scalar=1.0 / S_AR,
                            in1=h1h[:, qr * 4:(qr + 1) * 4, :],
                            op0=ALU.mult, op1=ALU.add)
                    halves.append(h1h)
                    sq8 = p5p.tile([P, DCH, SCW], e4, tag="p5",
                                   name=f"fsq8{hf}")
                    nc.vector.tensor_mul(sq8[:], h1h[:], h1h[:])
                    sqs.append(sq8)
                return halves, sqs

            def ffn_pro_mm(sc, halves, sqs):
                """Variance matmuls + rsqrt + normalize (muls on GpSimd so
                they overlap the previous chunk's wout dg matmuls)."""
                ss2 = psB.tile([P, SCW], f32, tag="pB", name="ss2_ps")
                for hf in range(2):
                    for jj in range(DCH // 2):
                        nc.tensor.matmul(
                            ss2[:], lhsT=ones4_sb[:],
                            rhs=sqs[hf][:, 2 * jj:2 * jj + 2, :],
                            start=(hf == 0 and jj == 0),
                            stop=(hf == 1 and jj == DCH // 2 - 1),
                            perf_mode=DR,
                        )
                rbc2 = rbp.tile([P, SCW], f32, tag="rbc", name="rbc2")
                nc.scalar.activation(
                    out=rbc2[:], in_=ss2[:], func=AF.Sqrt, bias=eps_t[:],
                    scale=1.0 / D,
                )
                nc.vector.reciprocal_approx_fast(out=rbc2[:], in_=rbc2[:])
                for d in range(DC):
                    y = halves[d // DCH][:, d % DCH, :]
                    nc.gpsimd.tensor_mul(y, y, rbc2[:])
                ffT = xarena.tile([P, FCC, SCW], bf, tag="x8", name="ffT")
                return ffT

            def ffn_fc(sc, fc, halves, ffT):
                w01b = wstr.tile([P, 2, DC, P], bf, tag="wstream", name="w01b")
                nc.sync.dma_start(out=w01b[:], in_=w01[fc])
                psg = psA.tile([P, SCW], f32, tag="pA", name="psg")
                psu = psA.tile([P, SCW], f32, tag="pA", name="psu")
                for d in range(DC):
                    y = halves[d // DCH][:, d % DCH, :]
                    nc.tensor.matmul(
                        psg[:], lhsT=w01b[:, 0, d, :], rhs=y,
                        start=(d == 0), stop=(d == DC - 1),
                    )
                    nc.tensor.matmul(
                        psu[:], lhsT=w01b[:, 1, d, :], rhs=y,
                        start=(d == 0), stop=(d == DC - 1),
                    )
                gel = stgp.tile([P, SCW], bf, tag="stg", name="gel")
                nc.scalar.activation(out=gel[:], in_=psg[:], func=AF.Gelu)
                nc.vector.tensor_mul(ffT[:, fc, :], psu[:], gel[:])

            def ffn_wout_dg(sc, dg, ffT):
                wob = wstr.tile([P, FCC, 4 * P], bf, tag="wstream", name="wob")
                nc.sync.dma_start(out=wob[:], in_=wout[dg])
                arp = stgp.tile([P, 4, SCW], e4, tag="arp", name="arp")
                nc.sync.dma_start(
                    out=arp[:],
                    in_=arin[sc][dg * 4:(dg + 1) * 4, :, :].rearrange(
                        "d p s -> p d s"),
                )
                stg = stgp.tile([P, 4, SCW], bf, tag="stg", name="ffstg")
                for di in range(4):
                    ps = psA.tile([P, SCW], f32, tag="pA", name="ps_o")
                    for fc in range(FCC):
                        nc.tensor.matmul(
                            ps[:], lhsT=wob[:, fc, di * P:(di + 1) * P],
                            rhs=ffT[:, fc, :],
                            start=(fc == 0), stop=(fc == FCC - 1),
                        )
                    nc.vector.scalar_tensor_tensor(
                        out=stg[:, di, :], in0=arp[:, di, :],
                        scalar=1.0 / S_AR, in1=ps[:],
                        op0=ALU.mult, op1=ALU.add)
                nc.sync.dma_start(
                    out=rsin[sc][dg * 4:(dg + 1) * 4, :, :].rearrange(
                        "d p s -> p d s"),
                    in_=stg[:],
                )

            def ffn_wout_dg3(dg, hf, ffT):
                """Last chunk, token-half hf: wout + attn-partial add."""
                tcols = slice(hf * SCH, (hf + 1) * SCH)
                wob = wstr.tile([P, FCC, 4 * P], bf, tag="wstream", name="wob")
                nc.sync.dma_start(out=wob[:], in_=wout[dg])
                arp = stgp.tile([P, 4, SCH], e4, tag="arp", name="arp3")
                nc.sync.dma_start(
                    out=arp[:],
                    in_=arin[3][dg * 4:(dg + 1) * 4, :, tcols].rearrange(
                        "d p s -> p d s"),
                )
                stg = stgp.tile([P, 4, SCH], bf, tag="stg", name="ffstg3")
                for di in range(4):
                    ps = psA.tile([P, SCH], f32, tag="pA", name="ps_o3")
                    for fc in range(FCC):
                        nc.tensor.matmul(
                            ps[:], lhsT=wob[:, fc, di * P:(di + 1) * P],
                            rhs=ffT[:, fc, tcols],
                            start=(fc == 0), stop=(fc == FCC - 1),
                        )
                    nc.vector.scalar_tensor_tensor(
                        out=stg[:, di, :], in0=arp[:, di, :],
                        scalar=1.0 / S_AR, in1=ps[:],
                        op0=ALU.mult, op1=ALU.add)
                nc.sync.dma_start(
                    out=rsin3[hf][dg * 4:(dg + 1) * 4, :, :].rearrange(
                        "d p s -> p d s"),
                    in_=stg[:],
                )

            def residual3(hf):
                cols = slice(3 * SCW + hf * SCH, 3 * SCW + (hf + 1) * SCH)
                rst = stgp.tile([P, 4, SCH], bf, tag="stg", name="rst3")
                nc.sync.dma_start(
                    out=rst[:], in_=rsout3[hf][:].rearrange("o p s -> p o s"))
                hot = stgp.tile([P, 4, SCH], bf, tag="stg", name="hot3")
                nc.sync.dma_start(
                    out=hot[:], in_=h_ownb[:, :, cols].rearrange("o p s -> p o s"))
                ot = stgp.tile([P, 4, SCH], bf, tag="ot", bufs=1, name="ot3")
                nc.vector.tensor_add(ot[:], hot[:], rst[:])
                nc.sync.dma_start(
                    out=out[:, cols].rearrange("(o p) s -> p o s", p=P),
                    in_=ot[:],
                )

            def residual(sc):
                cols = slice(sc * SCW, (sc + 1) * SCW)
                rst = stgp.tile([P, 4, SCW], bf, tag="stg", name="rst")
                nc.sync.dma_start(
                    out=rst[:], in_=rsout[sc][:].rearrange("o p s -> p o s"))
                hot = stgp.tile([P, 4, SCW], bf, tag="stg", name="hot")
                nc.sync.dma_start(
                    out=hot[:], in_=h_ownb[:, :, cols].rearrange("o p s -> p o s"))
                ot = stgp.tile([P, 4, SCW], bf, tag="ot", bufs=1, name="ot")
                nc.vector.tensor_add(ot[:], hot[:], rst[:])
                nc.sync.dma_start(
                    out=out[:, cols].rearrange("(o p) s -> p o s", p=P),
                    in_=ot[:],
                )

            # ================= emission schedule ===========================
            # collectives overlap attention (the lowest-DMA-demand phase);
            # FFN prologues straddle the wout blocks so the PE queue never
            # drains at chunk boundaries.
            qkv_pair(0)
            load_wo()
            qkv_pair(2)

            for qg in range(SCN):
                for u in attn_units(qg):
                    u()

            h, s = ffn_pro_dve(0)
            ffT = ffn_pro_mm(0, h, s)
            for sc in range(SCN):
                for fc in range(FCC):
                    ffn_fc(sc, fc, h, ffT)
                if sc < SCN - 1:
                    hn, sn = ffn_pro_dve(sc + 1)
                    for dg in range(8):
                        ffn_wout_dg(sc, dg, ffT)
                    ffTn = ffn_pro_mm(sc + 1, hn, sn)
                    nc.gpsimd.collective_compute(
                        "ReduceScatter", ALU.add, replica_groups=RG,
                        ins=[rsin[sc][:]], outs=[rsout[sc][:]],
                    )
                    h, ffT = hn, ffTn
                else:
                    # last chunk: wout + RS in two token halves so the tail
                    # collective is only 2MB
                    for hf in range(2):
                        for dg in range(8):
                            ffn_wout_dg3(dg, hf, ffT)
                        nc.gpsimd.collective_compute(
                            "ReduceScatter", ALU.add, replica_groups=RG,
                            ins=[rsin3[hf][:]], outs=[rsout3[hf][:]],
                        )
                if sc >= 1:
                    residual(sc - 1)
            residual3(0)
            residual3(1)

    nc.finalize()
    return nc


def _prep_in_maps(inputs):
    import ml_dtypes

    bf16 = ml_dtypes.bfloat16
    e4 = ml_dtypes.float8_e4m3
    e5 = ml_dtypes.float8_e5m2

    def q8(x, s):
        return np.clip(x * s, -240, 240).astype(e4)

    hid = np.ascontiguousarray(np.asarray(inputs["hidden_states"], np.float32)[0])
    mask = np.asarray(inputs["attention_mask"])[0]
    pbias = np.asarray(inputs["position_bias"], np.float32)[0]
    ln_a = np.asarray(inputs["ln_attn_w"], np.float32)
    ln_f = np.asarray(inputs["ln_ffn_w"], np.float32)
    wq = np.asarray(inputs["wq"], np.float32)
    wk = np.asarray(inputs["wk"], np.float32)
    wv = np.asarray(inputs["wv"], np.float32)
    wo = np.asarray(inputs["wo"], np.float32)
    w0 = np.asarray(inputs["w0"], np.float32)
    w1 = np.asarray(inputs["w1"], np.float32)
    w_out = np.asarray(inputs["w_out"], np.float32)

    hT = np.ascontiguousarray(hid.T)                          # (D, S) f32
    hT_bf = hT.reshape(DC, P, S).astype(bf16)
    wq_f = q8(ln_a[:, None] * wq * (DH ** -0.5), S_WQ)
    wk_f = q8(ln_a[:, None] * wk, S_WK)
    wv_f = q8(ln_a[:, None] * wv, S_WV)
    wo_f = q8(wo, S_WO)
    w0_f = (ln_f[:, None] * w0).astype(bf16)
    w1_f = (ln_f[:, None] * w1).astype(bf16)
    wout_f = w_out.astype(bf16)
    if mask.all():
        pb_m = pbias * S_QS
    else:
        pb_m = np.where(mask[None], pbias * S_QS, np.float32(-1e30))
    # transposed position bias: [H, S_k, S_q] -> per-core [HPC,SCN,2,P,8,SCW]
    pbT_full = np.clip(
        np.ascontiguousarray(pb_m.transpose(0, 2, 1)), -240, 240).astype(e4)

    ones4 = np.ones((P, 2, P), dtype=e4)
    ones5 = np.ones((P, 2, P), dtype=e5)

    def wqkv_layout(w):                # (D, WPC) -> (P, DCH, 2, WPC)
        return np.ascontiguousarray(
            w.reshape(DCH, 2, P, WPC).transpose(2, 0, 1, 3))

    in_maps = []
    for c in range(NCORES):
        ws = slice(c * WPC, (c + 1) * WPC)
        fs = slice(c * FPC, (c + 1) * FPC)
        wo_c = np.ascontiguousarray(
            wo_f[ws, :].reshape(2, 2, P, D).transpose(2, 0, 1, 3))
        pb_c = pbT_full[c * HPC:(c + 1) * HPC]                # (HPC, S_k, S_q)
        pb_c = pb_c.reshape(HPC, 2, 8, P, SCN, SCW).transpose(0, 4, 1, 3, 2, 5)
        w0_c = w0_f[:, fs].reshape(DC, P, FCC, P).transpose(2, 1, 0, 3)
        w1_c = w1_f[:, fs].reshape(DC, P, FCC, P).transpose(2, 1, 0, 3)
        w01_c = np.stack([w0_c, w1_c], axis=2)                # [FCC,P,2,DC,P]
        wout_c = wout_f[fs, :].reshape(FCC, P, 8, 4 * P).transpose(2, 1, 0, 3)
        in_maps.append({
            "hT": hT_bf,
            "h_ownb": np.ascontiguousarray(hT_bf[c * 4:(c + 1) * 4]),
            "wq": wqkv_layout(wq_f[:, ws]),
            "wk": wqkv_layout(wk_f[:, ws]),
            "wv": wqkv_layout(wv_f[:, ws]),
            "wo": wo_c,
            "pbT": np.ascontiguousarray(pb_c),
            "w01": np.ascontiguousarray(w01_c),
            "wout": np.ascontiguousarray(wout_c),
            "ones4": ones4,
            "ones5": ones5,
        })
    return in_maps


def get_nc():
    if "nc" not in _CACHE:
        _CACHE["nc"] = _build()
    return _CACHE["nc"]


def kernel(**inputs):
    from concourse.bass_utils import run_bass_kernel_spmd

    nc = get_nc()
    in_maps = _prep_in_maps(inputs)
    res = run_bass_kernel_spmd(nc, in_maps, core_ids=list(range(NCORES)))
    parts = [res.results[c]["out"] for c in range(NCORES)]   # each (WPC, S)
    full_T = np.concatenate(parts, axis=0)                    # (D, S)
    out = np.ascontiguousarray(full_T.T)[None]                # (1, S, D)
    return out.astype(np.float32)


# revision 36
# speedup vs baseline: 1.0530x; 1.0530x over previous
"""CPMAnt transformer block on 8 TRN2 NeuronCores (Megatron-style TP).

Core c owns 4 attention heads and 1280 FFN columns. Activations are
feature-major (D on partitions). QKV / attention-out / AV / softmax-sum /
sum-of-squares matmuls run in fp8 (e4m3 / e5m2) DoubleRow mode (2 k-tiles
per instruction = 2x PE throughput); scores and the FFN run in bf16.
q/k/v and attention probabilities never leave SBUF. Scores are computed
k-major (out[k, q]) so no PE transposes are needed; position bias is
stored e4m3 and added to the score PSUM on the DVE (no eye-matmul); the
softmax denominator comes from an fp8 ones-matmul and normalization is
folded into the attn output copy.

DMA-traffic cuts vs the first version: pb e4m3 (16.8MB), wq/wk/wv
streamed once per chunk-pair (12MB), wo streamed once (6MB), residual
base read as bf16 hT rows (14MB). FFN prologues are split into a DVE
part (h reload + attn-sum add + squares) emitted before the previous
chunk's wout units and an MM part (variance matmuls + normalize)
emitted after, so the in-order PE queue never drains at chunk
boundaries. The last chunk's wout+ReduceScatter run in two token
halves so the tail only waits on a 2MB collective.
"""

import math

import numpy as np

S = 2048
D = 4096
H = 32
DH = 128
FF = 10240
NCORES = 8
P = 128
HPC = H // NCORES            # 4 heads per core
WPC = HPC * DH               # 512   per-core qkv width
FPC = FF // NCORES           # 1280  per-core ff width
FCC = FPC // P               # 10
DC = D // P                  # 32
DCH = DC // 2                # 16
SCN = 4                      # S chunks
SCW = S // SCN               # 512
SCH = SCW // 2               # 256 (tail-split width)
KC = S // P                  # 16 key chunks
EPS = 1e-6

# fp8 weight scales (powers of two; descaled at psum copy-out)
S_WQ = 256.0                 # wq folded with 1/sqrt(DH): std ~0.0014
S_WK = 16.0
S_WV = 16.0
S_WO = 16.0
S_QS = 4.0                   # q stored as 4*q (e4m3); pb pre-scaled by 4 on host
S_VS = 8.0                   # v stored as 8*v; cancels with attn fp8 scale
S_AR = 256.0                 # attn-out partials AllReduced in e4m3 at this scale
                             # (small enough that the 8-way sum stays in range)
S_RS = 64.0                  # last chunk's (ffn+attn) partials ReduceScattered
                             # in e4m3; only 1/4 of tokens see the extra noise

_CACHE = {}


def _build():
    import concourse.bacc as bacc
    import concourse.tile as tile
    from concourse import mybir

    f32 = mybir.dt.float32
    bf = mybir.dt.bfloat16
    e4 = mybir.dt.float8e4
    e5 = mybir.dt.float8e5
    AF = mybir.ActivationFunctionType
    ALU = mybir.AluOpType
    DR = mybir.MatmulPerfMode.DoubleRow
    RG = [list(range(NCORES))]

    nc = bacc.Bacc(None, num_devices=NCORES)

    hT = nc.dram_tensor("hT", [DC, P, S], bf, kind="ExternalInput")
    h_ownb = nc.dram_tensor("h_ownb", [4, P, S], bf, kind="ExternalInput")
    wq = nc.dram_tensor("wq", [P, DCH, 2, WPC], e4, kind="ExternalInput")
    wk = nc.dram_tensor("wk", [P, DCH, 2, WPC], e4, kind="ExternalInput")
    wv = nc.dram_tensor("wv", [P, DCH, 2, WPC], e4, kind="ExternalInput")
    wo = nc.dram_tensor("wo", [P, 2, 2, D], e4, kind="ExternalInput")
    pbT = nc.dram_tensor("pbT", [HPC, SCN, 2, P, 8, SCW], e4, kind="ExternalInput")
    w01 = nc.dram_tensor("w01", [FCC, P, 2, DC, P], bf, kind="ExternalInput")
    wout = nc.dram_tensor("wout", [8, P, FCC, 4 * P], bf, kind="ExternalInput")
    ones4 = nc.dram_tensor("ones4", [P, 2, P], e4, kind="ExternalInput")
    ones5 = nc.dram_tensor("ones5", [P, 2, P], e5, kind="ExternalInput")
    out = nc.dram_tensor("out", [WPC, S], bf, kind="ExternalOutput")

    from contextlib import ExitStack

    with tile.TileContext(nc) as tc:
        with ExitStack() as ctx:
            ep = ctx.enter_context
            dram = ep(tc.tile_pool(name="dram", bufs=1, space="DRAM"))
            singles = ep(tc.tile_pool(name="singles", bufs=1))
            arena = ep(tc.tile_pool(name="arena", bufs=1))
            hstr = ep(tc.tile_pool(name="hstr", bufs=3))
            xarena = ep(tc.tile_pool(name="xarena", bufs=2))
            wstr = ep(tc.tile_pool(name="wstr", bufs=2))
            pbp = ep(tc.tile_pool(name="pbp", bufs=7))
            p5p = ep(tc.tile_pool(name="p5p", bufs=2))
            stgp = ep(tc.tile_pool(name="stgp", bufs=2))
            atp = ep(tc.tile_pool(name="atp", bufs=2))
            rbp = ep(tc.tile_pool(name="rbp", bufs=2))
            psA = ep(tc.tile_pool(name="psA", bufs=4, space="PSUM"))
            psB = ep(tc.tile_pool(name="psB", bufs=4, space="PSUM"))

            # ---- DRAM scratch for collectives ----
            # attn-out partials are tiny vs the residual stream, so the
            # AllReduce runs in e4m3 (half the wire bytes of bf16)
            arin = [dram.tile([DC, P, SCW], e4, tag=f"arin{j}", name=f"arin{j}")
                    for j in range(SCN)]
            arout = [dram.tile([DC, P, SCW], e4, tag=f"arout{j}", name=f"arout{j}",
                               addr_space="Shared") for j in range(SCN)]
            rsin = [dram.tile([DC, P, SCW], bf, tag=f"rsin{j}", name=f"rsin{j}")
                    for j in range(SCN - 1)]
            rsout = [dram.tile([4, P, SCW], bf, tag=f"rsout{j}", name=f"rsout{j}")
                     for j in range(SCN - 1)]
            # last chunk: two token-half RS so the tail collective is 2MB
            rsin3 = [dram.tile([DC, P, SCH], bf, tag=f"rsin3{h}", name=f"rsin3{h}")
                     for h in range(2)]
            rsout3 = [dram.tile([4, P, SCH], bf, tag=f"rsout3{h}", name=f"rsout3{h}")
                      for h in range(2)]

            ones4_sb = singles.tile([P, 2, P], e4)
            nc.sync.dma_start(out=ones4_sb[:], in_=ones4[:])
            ones5_sb = singles.tile([P, 2, P], e5)
            nc.sync.dma_start(out=ones5_sb[:], in_=ones5[:])
            eps_t = singles.tile([P, 1], f32)
            nc.vector.memset(eps_t[:], EPS)

            # persistent SBUF arenas for q/k/v (fp8)
            qT = arena.tile([P, HPC, S], e4, tag="qT")       # [dh, h, s] = 4*q
            kT = arena.tile([P, HPC, S], e4, tag="kT")       # [dh, h, s] = k
            v8 = arena.tile([P, HPC, 8, 2, DH], e4, tag="v8")  # 8*v

            # ================= phase 1: rmsnorm1 + QKV =====================
            def qkv_norm(j):
                """hT chunk -> normalized fp8 x8; returns x8."""
                cols = slice(j * SCW, (j + 1) * SCW)
                halves = []
                ss = psB.tile([P, SCW], f32, tag="pB", name="ss_ps")
                for hf in range(2):
                    hld = hstr.tile([P, DCH, SCW], bf, tag="hstream",
                                    name=f"hld{hf}")
                    nc.sync.dma_start(
                        out=hld[:],
                        in_=hT[hf * DCH:(hf + 1) * DCH, :, cols].rearrange(
                            "d p s -> p d s"),
                    )
                    halves.append(hld)
                    sq8 = p5p.tile([P, DCH, SCW], e4, tag="p5", name=f"sq8{hf}")
                    nc.vector.tensor_mul(sq8[:], hld[:], hld[:])
                    for jj in range(DCH // 2):
                        nc.tensor.matmul(
                            ss[:], lhsT=ones4_sb[:],
                            rhs=sq8[:, 2 * jj:2 * jj + 2, :],
                            start=(hf == 0 and jj == 0),
                            stop=(hf == 1 and jj == DCH // 2 - 1),
                            perf_mode=DR,
                        )
                rbc = rbp.tile([P, SCW], f32, tag="rbc")
                nc.scalar.activation(
                    out=rbc[:], in_=ss[:], func=AF.Sqrt, bias=eps_t[:],
                    scale=1.0 / D,
                )
                nc.vector.reciprocal_approx_fast(out=rbc[:], in_=rbc[:])
                x8 = xarena.tile([P, DC, SCW], e4, tag="x8")
                for d in range(DC):
                    nc.vector.tensor_mul(
                        x8[:, d, :], halves[d // DCH][:, d % DCH, :], rbc[:])
                return x8

            def qkv_pair(j0):
                """Two S-chunks per weight stream (wq/wk/wv loaded once)."""
                x0 = qkv_norm(j0)
                # wq load issued between the two norms so it lands in time
                wqsb = wstr.tile([P, DCH, 2, WPC], e4, tag="wstream",
                                 name="wqsb")
                nc.sync.dma_start(out=wqsb[:], in_=wq[:])
                xs = [(j0, x0), (j0 + 1, qkv_norm(j0 + 1))]
                for name, wsb_pre, wsrc, dst, cscale in (
                    ("q", wqsb, wq, qT, S_QS / S_WQ),
                    ("k", None, wk, kT, 1.0 / S_WK),
                ):
                    if wsb_pre is None:
                        wsb = wstr.tile([P, DCH, 2, WPC], e4, tag="wstream",
                                        name=f"w{name}sb")
                        nc.sync.dma_start(out=wsb[:], in_=wsrc[:])
                    else:
                        wsb = wsb_pre
                    for j, x8 in xs:
                        cols = slice(j * SCW, (j + 1) * SCW)
                        for h in range(HPC):
                            ps = psA.tile([P, SCW], f32, tag="pA",
                                          name=f"ps_{name}{h}")
                            for dp in range(DCH):
                                nc.tensor.matmul(
                                    ps[:], lhsT=wsb[:, dp, :, h * DH:(h + 1) * DH],
                                    rhs=x8[:, 2 * dp:2 * dp + 2, :],
                                    start=(dp == 0), stop=(dp == DCH - 1),
                                    perf_mode=DR,
                                )
                            nc.scalar.mul(dst[:, h, cols], ps[:], cscale)

                wvsb = wstr.tile([P, DCH, 2, WPC], e4, tag="wstream", name="wvsb")
                nc.sync.dma_start(out=wvsb[:], in_=wv[:])
                for j, x8 in xs:
                    for sl in range(SCW // P):
                        ps = psA.tile([P, WPC], f32, tag="pA", name=f"ps_v{sl}")
                        for dp in range(DCH):
                            nc.tensor.matmul(
                                ps[:], lhsT=x8[:, 2 * dp:2 * dp + 2, sl * P:(sl + 1) * P],
                                rhs=wvsb[:, dp, :, :],
                                start=(dp == 0), stop=(dp == DCH - 1),
                                perf_mode=DR,
                            )
                        kcix = j * (SCW // P) + sl
                        nc.scalar.mul(
                            v8[:, :, kcix // 2, kcix % 2, :],
                            ps[:].rearrange("p (h f) -> p h f", h=HPC),
                            S_VS / S_WV,
                        )

            # ================= attention units =============================
            def attn_scores(qg, h):
                qcols = slice(qg * SCW, (qg + 1) * SCW)
                p5 = p5p.tile([P, KC, SCW], e5, tag="p5", name="p5")
                for q4 in range(4):
                    pbt = pbp.tile([P, 4, SCW], e4, tag="pbt", name="pbt")
                    nc.sync.dma_start(
                        out=pbt[:],
                        in_=pbT[h, qg, q4 // 2][:, (q4 % 2) * 4:(q4 % 2) * 4 + 4, :],
                    )
                    for kk in range(4):
                        kc = q4 * 4 + kk
                        pss = psA.tile([P, SCW], f32, tag="pA", name="pss")
                        nc.tensor.matmul(
                            pss[:], lhsT=kT[:, h, kc * P:(kc + 1) * P],
                            rhs=qT[:, h, qcols], start=True, stop=True,
                        )
                        nc.vector.tensor_add(pss[:], pss[:], pbt[:, kk, :])
                        nc.scalar.activation(
                            out=p5[:, kc, :], in_=pss[:], func=AF.Exp,
                            scale=1.0 / S_QS,
                        )
                return p5

            def attn_sums_av(qg, h, p5, attnT):
                sums = psB.tile([P, SCW], f32, tag="pB", name="sums_ps")
                for jj in range(KC // 2):
                    nc.tensor.matmul(
                        sums[:], lhsT=ones5_sb[:],
                        rhs=p5[:, 2 * jj:2 * jj + 2, :],
                        start=(jj == 0), stop=(jj == KC // 2 - 1),
                        perf_mode=DR,
                    )
                psav = psB.tile([P, SCW], f32, tag="pB", name="psav")
                for jj in range(KC // 2):
                    nc.tensor.matmul(
                        psav[:], lhsT=v8[:, h, jj, :, :],
                        rhs=p5[:, 2 * jj:2 * jj + 2, :],
                        start=(jj == 0), stop=(jj == KC // 2 - 1),
                        perf_mode=DR,
                    )
                rs = rbp.tile([P, SCW], f32, tag="rbc", name="rs")
                nc.vector.reciprocal_approx_fast(out=rs[:], in_=sums[:])
                nc.vector.tensor_mul(attnT[:, h, :], psav[:], rs[:])

            wosb = None

            def load_wo():
                nonlocal wosb
                wosb = wstr.tile([P, 2, 2, D], e4, tag="wotag", bufs=1,
                                 name="wosb")
                nc.sync.dma_start(out=wosb[:], in_=wo[:])

            def wo_unit(qg, attnT):
                for dg in range(8):
                    stg = stgp.tile([P, 4, SCW], e4, tag="stg", name="wostg")
                    for di in range(4):
                        dcc = dg * 4 + di
                        ps = psA.tile([P, SCW], f32, tag="pA", name="ps_wo")
                        for hp in range(2):
                            nc.tensor.matmul(
                                ps[:],
                                lhsT=wosb[:, hp, :, dcc * P:(dcc + 1) * P],
                                rhs=attnT[:, 2 * hp:2 * hp + 2, :],
                                start=(hp == 0), stop=(hp == 1),
                                perf_mode=DR,
                            )
                        nc.vector.tensor_scalar_mul(
                            stg[:, di, :], ps[:], S_AR / (S_VS * S_WO))
                    nc.sync.dma_start(
                        out=arin[qg][dg * 4:(dg + 1) * 4, :, :].rearrange(
                            "d p s -> p d s"),
                        in_=stg[:],
                    )
                nc.gpsimd.collective_compute(
                    "AllReduce", ALU.add, replica_groups=RG,
                    ins=[arin[qg][:]], outs=[arout[qg][:]],
                )

            def attn_units(qg):
                """List of emit-callbacks for one attention chunk."""
                attnT = atp.tile([P, HPC, SCW], e4, tag="attnT",
                                 name=f"attnT{qg}")
                state = {}

                def u_scores(h):
                    def emit():
                        p5_prev = state.get("p5")
                        state["p5"] = attn_scores(qg, h)
                        if p5_prev is not None:
                            attn_sums_av(qg, h - 1, p5_prev, attnT)
                    return emit

                def u_tail():
                    def emit():
                        attn_sums_av(qg, HPC - 1, state["p5"], attnT)
                        wo_unit(qg, attnT)
                    return emit

                return [u_scores(h) for h in range(HPC)] + [u_tail()]

            # ================= FFN units ===================================
            def ffn_pro_dve(sc):
                """h reload + attn-sum add (DVE/DMA only)."""
                cols = slice(sc * SCW, (sc + 1) * SCW)
                halves = []
                for hf in range(2):
                    h1h = hstr.tile([P, DCH, SCW], bf, tag="hstream",
                                    name=f"h1h{hf}")
                    nc.sync.dma_start(
                        out=h1h[:],
                        in_=hT[hf * DCH:(hf + 1) * DCH, :, cols].rearrange(
                            "d p s -> p d s"),
                    )
                    for qr in range(4):
                        ars = stgp.tile([P, 4, SCW], e4, tag="arp", name="ars")
                        d0 = hf * DCH + qr * 4
                        nc.sync.dma_start(
                            out=ars[:],
                            in_=arout[sc][d0:d0 + 4, :, :].rearrange(
                                "d p s -> p d s"),
                        )
                        nc.vector.scalar_tensor_tensor(
                            out=h1h[:, qr * 4:(qr + 1) * 4, :],
                            in0=ars[:], scalar=1.0 / S_AR,
                            in1=h1h[:, qr * 4:(qr + 1) * 4, :],
                            op0=ALU.mult, op1=ALU.add)
                    halves.append(h1h)
                return halves

            def ffn_pro_mm(sc, halves):
                """Squares + variance matmuls + rsqrt + normalize (muls on
                GpSimd so they overlap the previous chunk's wout work)."""
                ss2 = psB.tile([P, SCW], f32, tag="pB", name="ss2_ps")
                for hf in range(2):
                    sq8 = p5p.tile([P, DCH, SCW], e4, tag="p5",
                                   name=f"fsq8{hf}")
                    nc.vector.tensor_mul(sq8[:], halves[hf][:], halves[hf][:])
                    for jj in range(DCH // 2):
                        nc.tensor.matmul(
                            ss2[:], lhsT=ones4_sb[:],
                            rhs=sq8[:, 2 * jj:2 * jj + 2, :],
                            start=(hf == 0 and jj == 0),
                            stop=(hf == 1 and jj == DCH // 2 - 1),
                            perf_mode=DR,
                        )
                rbc2 = rbp.tile([P, SCW], f32, tag="rbc", name="rbc2")
                nc.scalar.activation(
                    out=rbc2[:], in_=ss2[:], func=AF.Sqrt, bias=eps_t[:],
                    scale=1.0 / D,
                )
                nc.vector.reciprocal_approx_fast(out=rbc2[:], in_=rbc2[:])
                for d in range(DC):
                    y = halves[d // DCH][:, d % DCH, :]
                    nc.gpsimd.tensor_mul(y, y, rbc2[:])
                ffT = xarena.tile([P, FCC, SCW], bf, tag="x8", name="ffT")
                return ffT

            def ffn_fc(sc, fc, halves, ffT):
                w01b = wstr.tile([P, 2, DC, P], bf, tag="wstream", name="w01b")
                nc.sync.dma_start(out=w01b[:], in_=w01[fc])
                psg = psA.tile([P, SCW], f32, tag="pA", name="psg")
                psu = psA.tile([P, SCW], f32, tag="pA", name="psu")
                for d in range(DC):
                    y = halves[d // DCH][:, d % DCH, :]
                    nc.tensor.matmul(
                        psg[:], lhsT=w01b[:, 0, d, :], rhs=y,
                        start=(d == 0), stop=(d == DC - 1),
                    )
                    nc.tensor.matmul(
                        psu[:], lhsT=w01b[:, 1, d, :], rhs=y,
                        start=(d == 0), stop=(d == DC - 1),
                    )
                gel = stgp.tile([P, SCW], bf, tag="stg", name="gel")
                nc.scalar.activation(out=gel[:], in_=psg[:], func=AF.Gelu)
                nc.vector.tensor_mul(ffT[:, fc, :], psu[:], gel[:])

            def ffn_wout_dg(sc, dg, ffT):
                wob = wstr.tile([P, FCC, 4 * P], bf, tag="wstream", name="wob")
                nc.sync.dma_start(out=wob[:], in_=wout[dg])
                arp = stgp.tile([P, 4, SCW], e4, tag="arp", name="arp")
                nc.sync.dma_start(
                    out=arp[:],
                    in_=arin[sc][dg * 4:(dg + 1) * 4, :, :].rearrange(
                        "d p s -> p d s"),
                )
                stg = stgp.tile([P, 4, SCW], bf, tag="stg", name="ffstg")
                for di in range(4):
                    ps = psA.tile([P, SCW], f32, tag="pA", name="ps_o")
                    for fc in range(FCC):
                        nc.tensor.matmul(
                            ps[:], lhsT=wob[:, fc, di * P:(di + 1) * P],
                            rhs=ffT[:, fc, :],
                            start=(fc == 0), stop=(fc == FCC - 1),
                        )
                    nc.vector.scalar_tensor_tensor(
                        out=stg[:, di, :], in0=arp[:, di, :],
                        scalar=1.0 / S_AR, in1=ps[:],
                        op0=ALU.mult, op1=ALU.add)
                nc.sync.dma_start(
                    out=rsin[sc][dg * 4:(dg + 1) * 4, :, :].rearrange(
                        "d p s -> p d s"),
                    in_=stg[:],
                )

            def ffn_wout_dg3(dg, hf, ffT):
                """Last chunk, token-half hf: wout + attn-partial add."""
                tcols = slice(hf * SCH, (hf + 1) * SCH)
                wob = wstr.tile([P, FCC, 4 * P], bf, tag="wstream", name="wob")
                nc.sync.dma_start(out=wob[:], in_=wout[dg])
                arp = stgp.tile([P, 4, SCH], e4, tag="arp", name="arp3")
                nc.sync.dma_start(
                    out=arp[:],
                    in_=arin[3][dg * 4:(dg + 1) * 4, :, tcols].rearrange(
                        "d p s -> p d s"),
                )
                stg = stgp.tile([P, 4, SCH], bf, tag="stg", name="ffstg3")
                for di in range(4):
                    ps = psA.tile([P, SCH], f32, tag="pA", name="ps_o3")
                    for fc in range(FCC):
                        nc.tensor.matmul(
                            ps[:], lhsT=wob[:, fc, di * P:(di + 1) * P],
                            rhs=ffT[:, fc, tcols],
                            start=(fc == 0), stop=(fc == FCC - 1),
                        )
                    nc.vector.scalar_tensor_tensor(
                        out=stg[:, di, :], in0=arp[:, di, :],
                        scalar=1.0 / S_AR, in1=ps[:],
                        op0=ALU.mult, op1=ALU.add)
                nc.sync.dma_start(
                    out=rsin3[hf][dg * 4:(dg + 1) * 4, :, :].rearrange(
                        "d p s -> p d s"),
                    in_=stg[:],
                )

            def residual3(hf):
                cols = slice(3 * SCW + hf * SCH, 3 * SCW + (hf + 1) * SCH)
                rst = stgp.tile([P, 4, SCH], bf, tag="stg", name="rst3")
                nc.sync.dma_start(
                    out=rst[:], in_=rsout3[hf][:].rearrange("o p s -> p o s"))
                hot = stgp.tile([P, 4, SCH], bf, tag="stg", name="hot3")
                nc.sync.dma_start(
                    out=hot[:], in_=h_ownb[:, :, cols].rearrange("o p s -> p o s"))
                ot = stgp.tile([P, 4, SCH], bf, tag="ot", bufs=1, name="ot3")
                nc.vector.tensor_add(ot[:], hot[:], rst[:])
                nc.sync.dma_start(
                    out=out[:, cols].rearrange("(o p) s -> p o s", p=P),
                    in_=ot[:],
                )

            def residual(sc):
                cols = slice(sc * SCW, (sc + 1) * SCW)
                rst = stgp.tile([P, 4, SCW], bf, tag="stg", name="rst")
                nc.sync.dma_start(
                    out=rst[:], in_=rsout[sc][:].rearrange("o p s -> p o s"))
                hot = stgp.tile([P, 4, SCW], bf, tag="stg", name="hot")
                nc.sync.dma_start(
                    out=hot[:], in_=h_ownb[:, :, cols].rearrange("o p s -> p o s"))
                ot = stgp.tile([P, 4, SCW], bf, tag="ot", bufs=1, name="ot")
                nc.vector.tensor_add(ot[:], hot[:], rst[:])
                nc.sync.dma_start(
                    out=out[:, cols].rearrange("(o p) s -> p o s", p=P),
                    in_=ot[:],
                )

            # ================= emission schedule ===========================
            # collectives overlap attention (the lowest-DMA-demand phase);
            # FFN prologues straddle the wout blocks so the PE queue never
            # drains at chunk boundaries.
            qkv_pair(0)
            load_wo()
            qkv_pair(2)

            for qg in range(SCN):
                for u in attn_units(qg):
                    u()
                if qg == 2:
                    # chunk-0 h-reload + attn-sum adds hide inside attention
                    h = ffn_pro_dve(0)

            ffT = ffn_pro_mm(0, h)
            for sc in range(SCN):
                for fc in range(FCC):
                    ffn_fc(sc, fc, h, ffT)
                if sc < SCN - 1:
                    hn = ffn_pro_dve(sc + 1)
                    for dg in range(8):
                        ffn_wout_dg(sc, dg, ffT)
                    ffTn = ffn_pro_mm(sc + 1, hn)
                    nc.gpsimd.collective_compute(
                        "ReduceScatter", ALU.add, replica_groups=RG,
                        ins=[rsin[sc][:]], outs=[rsout[sc][:]],
                    )
                    h, ffT = hn, ffTn
                else:
                    # last chunk: wout + RS in two token halves so the tail
                    # collective is only 2MB
                    for hf in range(2):
                        for dg in range(8):
                            ffn_wout_dg3(dg, hf, ffT)
                        nc.gpsimd.collective_compute(
                            "ReduceScatter", ALU.add, replica_groups=RG,
                            ins=[rsin3[hf][:]], outs=[rsout3[hf][:]],
                        )
                if sc >= 1:
                    residual(sc - 1)
            residual3(0)
            residual3(1)

    nc.finalize()
    return nc


def _prep_in_maps(inputs):
    import ml_dtypes

    bf16 = ml_dtypes.bfloat16
    e4 = ml_dtypes.float8_e4m3
    e5 = ml_dtypes.float8_e5m2

    def q8(x, s):
        return np.clip(x * s, -240, 240).astype(e4)

    hid = np.ascontiguousarray(np.asarray(inputs["hidden_states"], np.float32)[0])
    mask = np.asarray(inputs["attention_mask"])[0]
    pbias = np.asarray(inputs["position_bias"], np.float32)[0]
    ln_a = np.asarray(inputs["ln_attn_w"], np.float32)
    ln_f = np.asarray(inputs["ln_ffn_w"], np.float32)
    wq = np.asarray(inputs["wq"], np.float32)
    wk = np.asarray(inputs["wk"], np.float32)
    wv = np.asarray(inputs["wv"], np.float32)
    wo = np.asarray(inputs["wo"], np.float32)
    w0 = np.asarray(inputs["w0"], np.float32)
    w1 = np.asarray(inputs["w1"], np.float32)
    w_out = np.asarray(inputs["w_out"], np.float32)

    hT = np.ascontiguousarray(hid.T)                          # (D, S) f32
    hT_bf = hT.reshape(DC, P, S).astype(bf16)
    wq_f = q8(ln_a[:, None] * wq * (DH ** -0.5), S_WQ)
    wk_f = q8(ln_a[:, None] * wk, S_WK)
    wv_f = q8(ln_a[:, None] * wv, S_WV)
    wo_f = q8(wo, S_WO)
    w0_f = (ln_f[:, None] * w0).astype(bf16)
    w1_f = (ln_f[:, None] * w1).astype(bf16)
    wout_f = w_out.astype(bf16)
    if mask.all():
        pb_m = pbias * S_QS
    else:
        pb_m = np.where(mask[None], pbias * S_QS, np.float32(-1e30))
    # transposed position bias: [H, S_k, S_q] -> per-core [HPC,SCN,2,P,8,SCW]
    pbT_full = np.clip(
        np.ascontiguousarray(pb_m.transpose(0, 2, 1)), -240, 240).astype(e4)

    ones4 = np.ones((P, 2, P), dtype=e4)
    ones5 = np.ones((P, 2, P), dtype=e5)

    def wqkv_layout(w):                # (D, WPC) -> (P, DCH, 2, WPC)
        return np.ascontiguousarray(
            w.reshape(DCH, 2, P, WPC).transpose(2, 0, 1, 3))

    in_maps = []
    for c in range(NCORES):
        ws = slice(c * WPC, (c + 1) * WPC)
        fs = slice(c * FPC, (c + 1) * FPC)
        wo_c = np.ascontiguousarray(
            wo_f[ws, :].reshape(2, 2, P, D).transpose(2, 0, 1, 3))
        pb_c = pbT_full[c * HPC:(c + 1) * HPC]                # (HPC, S_k, S_q)
        pb_c = pb_c.reshape(HPC, 2, 8, P, SCN, SCW).transpose(0, 4, 1, 3, 2, 5)
        w0_c = w0_f[:, fs].reshape(DC, P, FCC, P).transpose(2, 1, 0, 3)
        w1_c = w1_f[:, fs].reshape(DC, P, FCC, P).transpose(2, 1, 0, 3)
        w01_c = np.stack([w0_c, w1_c], axis=2)                # [FCC,P,2,DC,P]
        wout_c = wout_f[fs, :].reshape(FCC, P, 8, 4 * P).transpose(2, 1, 0, 3)
        in_maps.append({
            "hT": hT_bf,
            "h_ownb": np.ascontiguousarray(hT_bf[c * 4:(c + 1) * 4]),
            "wq": wqkv_layout(wq_f[:, ws]),
            "wk": wqkv_layout(wk_f[:, ws]),
            "wv": wqkv_layout(wv_f[:, ws]),
            "wo": wo_c,
            "pbT": np.ascontiguousarray(pb_c),
            "w01": np.ascontiguousarray(w01_c),
            "wout": np.ascontiguousarray(wout_c),
            "ones4": ones4,
            "ones5": ones5,
        })
    return in_maps


def get_nc():
    if "nc" not in _CACHE:
        _CACHE["nc"] = _build()
    return _CACHE["nc"]


def kernel(**inputs):
    from concourse.bass_utils import run_bass_kernel_spmd

    nc = get_nc()
    in_maps = _prep_in_maps(inputs)
    res = run_bass_kernel_spmd(nc, in_maps, core_ids=list(range(NCORES)))
    parts = [res.results[c]["out"] for c in range(NCORES)]   # each (WPC, S)
    full_T = np.concatenate(parts, axis=0)                    # (D, S)
    out = np.ascontiguousarray(full_T.T)[None]                # (1, S, D)
    return out.astype(np.float32)
